# revision 38
# baseline (speedup 1.0000x reference)
# DiabaticReadout forward on Trainium2 (Bass/Tile), 8-core data-parallel.
#
# Per sample i: H = [[d0, lam], [lam, d1]] (2x2 symmetric).  Eigenvalues in
# closed form:
#   mean    = 0.5*(d0+d1)
#   halfgap = sqrt(0.25*((d0-d1)^2 + 4*lam^2))
#   e0, e1  = mean -/+ halfgap          (ascending, matches eigh)
#
# Purely elementwise -> shard the N axis across the 8 NeuronCores, each core
# streams [128, F] tiles.  The kernel is HBM-bound at f32 (25 MB/core), and
# the rel-err budget (2e-2) dwarfs fp16 rounding (~1e-3 here), so ALL HBM
# I/O and all SBUF intermediates are fp16: half the DMA bytes, and every
# DVE tensor_tensor qualifies for the 2x_1p perf mode (2-byte dtype, packed
# innermost — measured 1114 ns vs 1846 ns per [128,2016] pass).
# scalar_tensor_tensor has NO perf mode (1x), so it is avoided entirely:
# the host folds the 0.5 factors into the fp16 quantization (ships d0/2,
# d1/2, lam/2 — an exact exponent shift), making sum = mean directly and
# sqrt scale-free.  The 3 inputs are host-packed into one [128, 3, rows]
# tensor per core and e0/e1 into one [128, 2, rows] output, so each tile is
# ONE load DMA + ONE store DMA (one issue + one completion semaphore each).
#
# Measured balance per [128, 2016] tile: DMA 2.58 MB (~6.2 us at the
# ~400 GB/s/core the DGE sustains), DVE 5 TT passes (~5.6 us), ACT 3 passes
# (~5.5 us) — three-way balanced; total is stream (~30 us) + fixed NEFF
# pre/postamble (~16 us: engine barriers + iram load up front, a ~6 us
# runtime semaphore-reset storm at the end) + DMA ramp/drain (~7 us).
# Loads issue from the SP HWDGE ring, stores from the GPSIMD SWDGE ring so
# neither queues behind the other.

import numpy as np

import concourse.bacc as bacc_mod
import concourse.tile as tile
from concourse import bacc, mybir
from concourse.bass_utils import run_bass_kernel_spmd

import contextlib


@contextlib.contextmanager
def _pin_act_table(keep="sqrt_and_others"):
    """All our activations (Square, Sqrt, Copy) live in the single
    `sqrt_and_others` set, but the table-load pass greedily picks the first
    set containing each function, which alternates tables per tile
    (~2.5us/tile of ACT_TABLE_LOAD thrash).  Present every other set as
    empty during compile so the pass pins everything to one table; indices
    stay aligned with act_info.json."""
    orig = bacc_mod.get_activation_tables

    def patched(arch):
        t = orig(arch)
        assert keep in t, sorted(t)
        return {name: (funcs if name == keep else set()) for name, funcs in t.items()}

    bacc_mod.get_activation_tables = patched
    try:
        yield
    finally:
        bacc_mod.get_activation_tables = orig

N_CORES = 8
P = 128  # SBUF partitions

_cache = {}


def _tile_schedule(rows, f_tile, ramp, ramp_end=()):
    """Tile-size schedule: optional small prologue/epilogue tiles so the
    pipeline fills/drains quickly, f_tile-sized tiles in the middle."""
    head, tail = [], []
    left = rows
    for s in ramp:
        if left <= 0:
            break
        s = min(s, left)
        head.append(s)
        left -= s
    for s in ramp_end:
        if left <= 0:
            break
        s = min(s, left)
        tail.append(s)
        left -= s
    mid = []
    if left > 0:
        k = -(-left // f_tile)  # ceil: number of mid tiles
        base = left // k
        extra = left - base * k  # first `extra` tiles get +1 row
        mid = [base + 1] * extra + [base] * (k - extra)
    return head + mid + tail[::-1]


def _build(rows, f_tile=2016, in_bufs=4, out_bufs=6, tmp_bufs=3,
           sum_engine="vector", store_engine="gpsimd", e1_engine="vector",
           e1_store_engine=None, s_engine="vector", lam_engine="sync",
           d2_engine="scalar", dif_first=True, alias_tmps=True, hs_inplace=True,
           prescale=True, packed="tiled", ramp=(), ramp_end=(512,),
           load_queues=("sync",), store_queues=("sync",),
           loads_first=True, io_dtype="float16"):
    """Build the per-core Bass module.  packed=True: one input tensor
    din [P, 3, rows] (d0|d1|lam per partition) and one output tensor
    eout [P, 2, rows] (e0|e1), so each tile needs ONE load DMA and ONE
    store DMA (one issue + one completion semaphore instead of 3+2);
    the host does the (free) interleaving."""
    C = P * rows
    fio = getattr(mybir.dt, io_dtype)
    Alu = mybir.AluOpType
    Act = mybir.ActivationFunctionType

    nc = bacc.Bacc(
        "TRN2",
        target_bir_lowering=False,
        debug=False,
        num_devices=N_CORES,
    )
    if packed == "tiled":
        # host lays out per-partition, per-tile contiguous [3, F] input
        # blocks and [2, F] output blocks: one DMA descriptor run of
        # 3F/2F elements per partition per tile.
        din = nc.dram_tensor("din", [3 * C], fio, kind="ExternalInput").ap()
        eout = nc.dram_tensor("eout", [2 * C], fio, kind="ExternalOutput").ap()
        dinv = din.rearrange("(p f) -> p f", p=P)
        eoutv = eout.rearrange("(p f) -> p f", p=P)
    elif packed:
        din = nc.dram_tensor("din", [3 * C], fio, kind="ExternalInput").ap()
        eout = nc.dram_tensor("eout", [2 * C], fio, kind="ExternalOutput").ap()
        dinv = din.rearrange("(p three f) -> p three f", p=P, three=3)
        eoutv = eout.rearrange("(p two f) -> p two f", p=P, two=2)
    else:
        d0 = nc.dram_tensor("d0", [C], fio, kind="ExternalInput").ap()
        d1 = nc.dram_tensor("d1", [C], fio, kind="ExternalInput").ap()
        lam = nc.dram_tensor("lam", [C], fio, kind="ExternalInput").ap()
        e0 = nc.dram_tensor("e0", [C], fio, kind="ExternalOutput").ap()
        e1 = nc.dram_tensor("e1", [C], fio, kind="ExternalOutput").ap()
        d0v = d0.rearrange("(p f) -> p f", p=P)
        d1v = d1.rearrange("(p f) -> p f", p=P)
        lamv = lam.rearrange("(p f) -> p f", p=P)
        e0v = e0.rearrange("(p f) -> p f", p=P)
        e1v = e1.rearrange("(p f) -> p f", p=P)

    sum_eng = getattr(nc, sum_engine)
    store_eng = getattr(nc, store_engine)
    e1_store_eng = getattr(nc, e1_store_engine) if e1_store_engine else store_eng
    e1_eng = getattr(nc, e1_engine)
    s_eng = getattr(nc, s_engine)
    # Each DGE queue tops out at ~210 GB/s (16 engines x ~13 GB/s); spread
    # loads/stores across queues round-robin by tile so the aggregate
    # ~435 GB/s is reachable.
    load_engs = [getattr(nc, e) for e in load_queues]
    store_engs = [getattr(nc, e) for e in store_queues]
    sizes = _tile_schedule(rows, f_tile, ramp, ramp_end)

    with tile.TileContext(nc) as tc:
        with (
            tc.tile_pool(name="ins", bufs=in_bufs) as ins,
            tc.tile_pool(name="outs", bufs=out_bufs) as outs,
            tc.tile_pool(name="tmp", bufs=tmp_bufs) as tmp,
        ):
            def _load(ti, F, f0):
                load_eng = load_engs[ti % len(load_engs)]
                sl = slice(f0, f0 + F)
                if packed == "tiled":
                    t_in = ins.tile([P, 3 * F], fio, tag="in")
                    load_eng.dma_start(t_in[:], dinv[:, 3 * f0 : 3 * f0 + 3 * F])
                    return t_in[:, 0:F], t_in[:, F : 2 * F], t_in[:, 2 * F : 3 * F]
                elif packed:
                    t_in = ins.tile([P, 3, F], fio, tag="in")
                    load_eng.dma_start(t_in[:], dinv[:, :, sl])
                    return t_in[:, 0, :], t_in[:, 1, :], t_in[:, 2, :]
                else:
                    t_d0 = ins.tile([P, F], fio, tag="d0")
                    nc.sync.dma_start(t_d0[:], d0v[:, sl])
                    t_d1 = ins.tile([P, F], fio, tag="d1")
                    nc.sync.dma_start(t_d1[:], d1v[:, sl])
                    t_lam = ins.tile([P, F], fio, tag="lam")
                    getattr(nc, lam_engine).dma_start(t_lam[:], lamv[:, sl])
                    return t_d0[:], t_d1[:], t_lam[:]

            offs = []
            f0 = 0
            for F in sizes:
                offs.append(f0)
                f0 += F
            loaded = {}
            if loads_first:
                # issue every load up front so no load-issue ever queues
                # behind a compute-dependent store issue on the same engine
                for ti, F in enumerate(sizes):
                    loaded[ti] = _load(ti, F, offs[ti])

            for ti, F in enumerate(sizes):
                f0 = offs[ti]
                sl = slice(f0, f0 + F)
                st_eng = store_engs[ti % len(store_engs)]
                t_d0, t_d1, t_lam = (
                    loaded[ti] if loads_first else _load(ti, F, f0)
                )

                # dif feeds the critical path (dif -> d2 -> s -> sqrt);
                # sum only feeds the final two output ops.
                t_sum = tmp.tile([P, F], fio, tag="sum")
                t_dif = tmp.tile([P, F], fio, tag="dif")
                if dif_first:
                    nc.vector.tensor_sub(t_dif[:], t_d0, t_d1)
                    sum_eng.tensor_add(t_sum[:], t_d0, t_d1)
                else:
                    sum_eng.tensor_add(t_sum[:], t_d0, t_d1)
                    nc.vector.tensor_sub(t_dif[:], t_d0, t_d1)

                # With prescale the host sends d0/2, d1/2, lam/2, so
                # sum = m (the mean) directly and dif = delta; then
                # l2 = (2*(lam/2))^2 = lam^2, d2 = delta^2,
                # s = delta^2 + lam^2, r = sqrt(s) (no scale), e = m -/+ r.
                t_l2 = tmp.tile([P, F], fio, tag="l2")
                nc.scalar.activation(t_l2[:], t_lam, Act.Square, scale=2.0)
                t_d2 = tmp.tile([P, F], fio, tag="dif" if alias_tmps else "d2")
                if d2_engine == "scalar":
                    nc.scalar.activation(t_d2[:], t_dif[:], Act.Square)
                else:
                    getattr(nc, d2_engine).tensor_mul(t_d2[:], t_dif[:], t_dif[:])

                t_s = tmp.tile([P, F], fio, tag="l2" if alias_tmps else "s")
                s_eng.tensor_add(t_s[:], t_d2[:], t_l2[:])
                t_r = tmp.tile([P, F], fio, tag="dif" if alias_tmps else "r")
                nc.scalar.activation(
                    t_r[:], t_s[:], Act.Sqrt, scale=(1.0 if prescale else 0.25)
                )

                if prescale:
                    t_hs = t_sum
                else:
                    # hs = 0.5*sum (tensor_scalar: 4x perf mode)
                    if hs_inplace:
                        t_hs = t_sum
                    else:
                        t_hs = tmp.tile([P, F], fio, tag="hs")
                    nc.vector.tensor_scalar_mul(t_hs[:], t_sum[:], 0.5)

                if packed == "tiled":
                    t_eo = outs.tile([P, 2 * F], fio, tag="eo")
                    nc.vector.tensor_sub(t_eo[:, 0:F], t_hs[:], t_r[:])
                    e1_eng.tensor_add(t_eo[:, F : 2 * F], t_hs[:], t_r[:])
                    st_eng.dma_start(eoutv[:, 2 * f0 : 2 * f0 + 2 * F], t_eo[:])
                elif packed:
                    t_eo = outs.tile([P, 2, F], fio, tag="eo")
                    nc.vector.tensor_sub(t_eo[:, 0, :], t_hs[:], t_r[:])
                    e1_eng.tensor_add(t_eo[:, 1, :], t_hs[:], t_r[:])
                    st_eng.dma_start(eoutv[:, :, sl], t_eo[:])
                else:
                    t_e0 = outs.tile([P, F], fio, tag="e0")
                    nc.vector.tensor_sub(t_e0[:], t_hs[:], t_r[:])
                    t_e1 = outs.tile([P, F], fio, tag="e1")
                    e1_eng.tensor_add(t_e1[:], t_hs[:], t_r[:])
                    store_eng.dma_start(e0v[:, sl], t_e0[:])
                    e1_store_eng.dma_start(e1v[:, sl], t_e1[:])

                f0 += F
    with _pin_act_table():
        nc.compile()
    return nc


def _get_nc(rows, **cfg):
    for k in ("ramp", "ramp_end", "load_queues", "store_queues"):
        if k in cfg:
            cfg[k] = tuple(cfg[k])
    key = (rows, tuple(sorted(cfg.items())))
    if key not in _cache:
        _cache[key] = _build(rows, **cfg)
    return _cache[key]


def kernel(d0, d1, lam, _trace=False, **cfg):
    np_io = np.dtype(cfg.get("io_dtype", "float16"))
    # prescale: ship d/2 and lam/2 (exact power-of-2 scale, folded into the
    # fp16 quantization) so the device computes m = d0'+d1' without a
    # separate 0.5x pass.
    sc = np.float32(0.5) if cfg.get("prescale", True) else np.float32(1.0)
    d0 = (np.asarray(d0, dtype=np.float32).ravel() * sc).astype(np_io)
    d1 = (np.asarray(d1, dtype=np.float32).ravel() * sc).astype(np_io)
    lam = (np.asarray(lam, dtype=np.float32).ravel() * sc).astype(np_io)
    n = d0.shape[0]

    # Per-core sample count: multiple of 128, cores cover ceil(n / 8).
    rows = -(-n // (N_CORES * P))  # ceil
    C = P * rows
    total = N_CORES * C
    pad = total - n
    if pad:
        z = np.zeros(pad, np_io)
        d0 = np.concatenate([d0, z])
        d1 = np.concatenate([d1, z])
        lam = np.concatenate([lam, z])

    packed = cfg.get("packed", "tiled")
    if packed == "tiled":
        # per core: per-partition, per-tile contiguous [3, F_i] blocks
        sizes = _tile_schedule(
            rows,
            cfg.get("f_tile", 2016),
            tuple(cfg.get("ramp", ())),
            tuple(cfg.get("ramp_end", (512,))),
        )
        in_maps = []
        for c in range(N_CORES):
            blk = np.empty((P, 3 * rows), np_io)
            d0c = d0[c * C : (c + 1) * C].reshape(P, rows)
            d1c = d1[c * C : (c + 1) * C].reshape(P, rows)
            lamc = lam[c * C : (c + 1) * C].reshape(P, rows)
            f0 = 0
            for F in sizes:
                b = 3 * f0
                blk[:, b : b + F] = d0c[:, f0 : f0 + F]
                blk[:, b + F : b + 2 * F] = d1c[:, f0 : f0 + F]
                blk[:, b + 2 * F : b + 3 * F] = lamc[:, f0 : f0 + F]
                f0 += F
            in_maps.append({"din": blk.ravel()})
    elif packed:
        # per core: [P, 3, rows] with [:, 0]=d0, [:, 1]=d1, [:, 2]=lam
        in_maps = []
        for c in range(N_CORES):
            blk = np.empty((P, 3, rows), np_io)
            blk[:, 0, :] = d0[c * C : (c + 1) * C].reshape(P, rows)
            blk[:, 1, :] = d1[c * C : (c + 1) * C].reshape(P, rows)
            blk[:, 2, :] = lam[c * C : (c + 1) * C].reshape(P, rows)
            in_maps.append({"din": blk.ravel()})
    else:
        in_maps = [
            {
                "d0": np.ascontiguousarray(d0[c * C : (c + 1) * C]),
                "d1": np.ascontiguousarray(d1[c * C : (c + 1) * C]),
                "lam": np.ascontiguousarray(lam[c * C : (c + 1) * C]),
            }
            for c in range(N_CORES)
        ]

    nc = _get_nc(rows, **cfg)
    res = run_bass_kernel_spmd(
        nc, in_maps, core_ids=list(range(N_CORES)), trace=_trace
    )
    global last_results
    last_results = res
    if packed == "tiled":
        full_e0 = np.empty((N_CORES, P, rows), np_io)
        full_e1 = np.empty((N_CORES, P, rows), np_io)
        for c in range(N_CORES):
            eo = res.results[c]["eout"].reshape(P, 2 * rows)
            f0 = 0
            for F in sizes:
                b = 2 * f0
                full_e0[c, :, f0 : f0 + F] = eo[:, b : b + F]
                full_e1[c, :, f0 : f0 + F] = eo[:, b + F : b + 2 * F]
                f0 += F
        full_e0 = full_e0.reshape(-1)
        full_e1 = full_e1.reshape(-1)
    elif packed:
        eo = np.stack(
            [res.results[c]["eout"].reshape(P, 2, rows) for c in range(N_CORES)]
        )  # [N_CORES, P, 2, rows]
        full_e0 = eo[:, :, 0, :].reshape(-1)
        full_e1 = eo[:, :, 1, :].reshape(-1)
    else:
        full_e0 = np.concatenate([res.results[c]["e0"] for c in range(N_CORES)])
        full_e1 = np.concatenate([res.results[c]["e1"] for c in range(N_CORES)])
    out = np.empty((n, 2), dtype=np.float32)
    out[:, 0] = full_e0[:n]
    out[:, 1] = full_e1[:n]
    return out


last_results = None


# revision 41
# speedup vs baseline: 1.0096x; 1.0096x over previous
# DiabaticReadout forward on Trainium2 (Bass/Tile), 8-core data-parallel.
#
# Per sample i: H = [[d0, lam], [lam, d1]] (2x2 symmetric).  Eigenvalues in
# closed form:
#   mean    = 0.5*(d0+d1)
#   halfgap = sqrt(0.25*((d0-d1)^2 + 4*lam^2))
#   e0, e1  = mean -/+ halfgap          (ascending, matches eigh)
#
# Purely elementwise -> shard the N axis across the 8 NeuronCores, each core
# streams [128, F] tiles.  The kernel is HBM-bound at f32 (25 MB/core), and
# the rel-err budget (2e-2) dwarfs fp16 rounding (~1e-3 here), so ALL HBM
# I/O and all SBUF intermediates are fp16: half the DMA bytes, and every
# DVE tensor_tensor qualifies for the 2x_1p perf mode (2-byte dtype, packed
# innermost — measured 1114 ns vs 1846 ns per [128,2016] pass).
# scalar_tensor_tensor has NO perf mode (1x), so it is avoided entirely:
# the host folds the 0.5 factors into the fp16 quantization (ships d0/2,
# d1/2, lam/2 — an exact exponent shift), making sum = mean directly and
# sqrt scale-free.  The 3 inputs are host-packed into one [128, 3, rows]
# tensor per core and e0/e1 into one [128, 2, rows] output, so each tile is
# ONE load DMA + ONE store DMA (one issue + one completion semaphore each).
#
# packed="tiled" additionally lays the host data out per-tile so every
# partition's tile chunk is one contiguous 3F*2-byte run: the DGE then
# moves ~11 KB packets (~26.5 GB/s per DMA engine) instead of 4 KB ones,
# lifting the single queue from ~210 to ~310 GB/s.  All loads are issued
# UP FRONT (loads_first) so no load issue ever queues behind a
# compute-dependent store issue, and stores ride the same SP HWDGE queue
# (SWDGE store packets were observed to lag their issue by ~14 us).
#
# Measured balance per [128, ~1851] tile: DVE 5 TT passes and ACT 3 passes
# ~5.2 us each, DMA ~5.9 us — three-way balanced; total is stream (~30 us,
# both engines >90% occupied) + fixed NEFF pre/postamble (~16 us: engine
# barriers + iram load up front, a ~6 us runtime semaphore-reset storm at
# the end) + DMA ramp/drain (~6 us).

import numpy as np

import concourse.bacc as bacc_mod
import concourse.tile as tile
from concourse import bacc, mybir
from concourse.bass_utils import run_bass_kernel_spmd

import contextlib


@contextlib.contextmanager
def _pin_act_table(keep="sqrt_and_others"):
    """All our activations (Square, Sqrt, Copy) live in the single
    `sqrt_and_others` set, but the table-load pass greedily picks the first
    set containing each function, which alternates tables per tile
    (~2.5us/tile of ACT_TABLE_LOAD thrash).  Present every other set as
    empty during compile so the pass pins everything to one table; indices
    stay aligned with act_info.json."""
    orig = bacc_mod.get_activation_tables

    def patched(arch):
        t = orig(arch)
        assert keep in t, sorted(t)
        return {name: (funcs if name == keep else set()) for name, funcs in t.items()}

    bacc_mod.get_activation_tables = patched
    try:
        yield
    finally:
        bacc_mod.get_activation_tables = orig

N_CORES = 8
P = 128  # SBUF partitions

_cache = {}


def _tile_schedule(rows, f_tile, ramp, ramp_end=()):
    """Tile-size schedule: optional small prologue/epilogue tiles so the
    pipeline fills/drains quickly, f_tile-sized tiles in the middle."""
    head, tail = [], []
    left = rows
    for s in ramp:
        if left <= 0:
            break
        s = min(s, left)
        head.append(s)
        left -= s
    for s in ramp_end:
        if left <= 0:
            break
        s = min(s, left)
        tail.append(s)
        left -= s
    mid = []
    if left > 0:
        k = -(-left // f_tile)  # ceil: number of mid tiles
        base = left // k
        extra = left - base * k  # first `extra` tiles get +1 row
        mid = [base + 1] * extra + [base] * (k - extra)
    return head + mid + tail[::-1]


def _build(rows, f_tile=2016, in_bufs=4, out_bufs=6, tmp_bufs=3,
           sum_engine="vector", store_engine="gpsimd", e1_engine="vector",
           e1_store_engine=None, s_engine="vector", lam_engine="sync",
           d2_engine="scalar", dif_first=True, alias_tmps=True, hs_inplace=True,
           prescale=True, packed="tiled", ramp=(), ramp_end=(512,),
           load_queues=("sync",), store_queues=("sync",),
           loads_first=True, split_lam=False, io_dtype="float16"):
    """Build the per-core Bass module.  packed=True: one input tensor
    din [P, 3, rows] (d0|d1|lam per partition) and one output tensor
    eout [P, 2, rows] (e0|e1), so each tile needs ONE load DMA and ONE
    store DMA (one issue + one completion semaphore instead of 3+2);
    the host does the (free) interleaving."""
    C = P * rows
    fio = getattr(mybir.dt, io_dtype)
    Alu = mybir.AluOpType
    Act = mybir.ActivationFunctionType

    nc = bacc.Bacc(
        "TRN2",
        target_bir_lowering=False,
        debug=False,
        num_devices=N_CORES,
    )
    if packed == "tiled":
        # host lays out per-partition, per-tile contiguous [3, F] input
        # blocks and [2, F] output blocks: one DMA descriptor run of
        # 3F/2F elements per partition per tile.
        din = nc.dram_tensor("din", [3 * C], fio, kind="ExternalInput").ap()
        eout = nc.dram_tensor("eout", [2 * C], fio, kind="ExternalOutput").ap()
        dinv = din.rearrange("(p f) -> p f", p=P)
        eoutv = eout.rearrange("(p f) -> p f", p=P)
    elif packed:
        din = nc.dram_tensor("din", [3 * C], fio, kind="ExternalInput").ap()
        eout = nc.dram_tensor("eout", [2 * C], fio, kind="ExternalOutput").ap()
        dinv = din.rearrange("(p three f) -> p three f", p=P, three=3)
        eoutv = eout.rearrange("(p two f) -> p two f", p=P, two=2)
    else:
        d0 = nc.dram_tensor("d0", [C], fio, kind="ExternalInput").ap()
        d1 = nc.dram_tensor("d1", [C], fio, kind="ExternalInput").ap()
        lam = nc.dram_tensor("lam", [C], fio, kind="ExternalInput").ap()
        e0 = nc.dram_tensor("e0", [C], fio, kind="ExternalOutput").ap()
        e1 = nc.dram_tensor("e1", [C], fio, kind="ExternalOutput").ap()
        d0v = d0.rearrange("(p f) -> p f", p=P)
        d1v = d1.rearrange("(p f) -> p f", p=P)
        lamv = lam.rearrange("(p f) -> p f", p=P)
        e0v = e0.rearrange("(p f) -> p f", p=P)
        e1v = e1.rearrange("(p f) -> p f", p=P)

    sum_eng = getattr(nc, sum_engine)
    store_eng = getattr(nc, store_engine)
    e1_store_eng = getattr(nc, e1_store_engine) if e1_store_engine else store_eng
    e1_eng = getattr(nc, e1_engine)
    s_eng = getattr(nc, s_engine)
    # Each DGE queue tops out at ~210 GB/s (16 engines x ~13 GB/s); spread
    # loads/stores across queues round-robin by tile so the aggregate
    # ~435 GB/s is reachable.
    load_engs = [getattr(nc, e) for e in load_queues]
    store_engs = [getattr(nc, e) for e in store_queues]
    sizes = _tile_schedule(rows, f_tile, ramp, ramp_end)

    with tile.TileContext(nc) as tc:
        with (
            tc.tile_pool(name="ins", bufs=in_bufs) as ins,
            tc.tile_pool(name="outs", bufs=out_bufs) as outs,
            tc.tile_pool(name="tmp", bufs=tmp_bufs) as tmp,
        ):
            def _load(ti, F, f0):
                load_eng = load_engs[ti % len(load_engs)]
                sl = slice(f0, f0 + F)
                if packed == "tiled":
                    t_in = ins.tile([P, 3 * F], fio, tag="in")
                    if split_lam:
                        # d0|d1 land first so sub/sum never wait on lam
                        load_eng.dma_start(
                            t_in[:, 0 : 2 * F], dinv[:, 3 * f0 : 3 * f0 + 2 * F]
                        )
                        load_eng.dma_start(
                            t_in[:, 2 * F : 3 * F],
                            dinv[:, 3 * f0 + 2 * F : 3 * f0 + 3 * F],
                        )
                    else:
                        load_eng.dma_start(t_in[:], dinv[:, 3 * f0 : 3 * f0 + 3 * F])
                    return t_in[:, 0:F], t_in[:, F : 2 * F], t_in[:, 2 * F : 3 * F]
                elif packed:
                    t_in = ins.tile([P, 3, F], fio, tag="in")
                    load_eng.dma_start(t_in[:], dinv[:, :, sl])
                    return t_in[:, 0, :], t_in[:, 1, :], t_in[:, 2, :]
                else:
                    t_d0 = ins.tile([P, F], fio, tag="d0")
                    nc.sync.dma_start(t_d0[:], d0v[:, sl])
                    t_d1 = ins.tile([P, F], fio, tag="d1")
                    nc.sync.dma_start(t_d1[:], d1v[:, sl])
                    t_lam = ins.tile([P, F], fio, tag="lam")
                    getattr(nc, lam_engine).dma_start(t_lam[:], lamv[:, sl])
                    return t_d0[:], t_d1[:], t_lam[:]

            offs = []
            f0 = 0
            for F in sizes:
                offs.append(f0)
                f0 += F
            loaded = {}
            if loads_first:
                # issue every load up front so no load-issue ever queues
                # behind a compute-dependent store issue on the same engine
                for ti, F in enumerate(sizes):
                    loaded[ti] = _load(ti, F, offs[ti])

            for ti, F in enumerate(sizes):
                f0 = offs[ti]
                sl = slice(f0, f0 + F)
                st_eng = store_engs[ti % len(store_engs)]
                t_d0, t_d1, t_lam = (
                    loaded[ti] if loads_first else _load(ti, F, f0)
                )

                # dif feeds the critical path (dif -> d2 -> s -> sqrt);
                # sum only feeds the final two output ops.
                t_sum = tmp.tile([P, F], fio, tag="sum")
                t_dif = tmp.tile([P, F], fio, tag="dif")
                if dif_first:
                    nc.vector.tensor_sub(t_dif[:], t_d0, t_d1)
                    sum_eng.tensor_add(t_sum[:], t_d0, t_d1)
                else:
                    sum_eng.tensor_add(t_sum[:], t_d0, t_d1)
                    nc.vector.tensor_sub(t_dif[:], t_d0, t_d1)

                # With prescale the host sends d0/2, d1/2, lam/2, so
                # sum = m (the mean) directly and dif = delta; then
                # l2 = (2*(lam/2))^2 = lam^2, d2 = delta^2,
                # s = delta^2 + lam^2, r = sqrt(s) (no scale), e = m -/+ r.
                t_l2 = tmp.tile([P, F], fio, tag="l2")
                nc.scalar.activation(t_l2[:], t_lam, Act.Square, scale=2.0)
                t_d2 = tmp.tile([P, F], fio, tag="dif" if alias_tmps else "d2")
                if d2_engine == "scalar":
                    nc.scalar.activation(t_d2[:], t_dif[:], Act.Square)
                else:
                    getattr(nc, d2_engine).tensor_mul(t_d2[:], t_dif[:], t_dif[:])

                t_s = tmp.tile([P, F], fio, tag="l2" if alias_tmps else "s")
                s_eng.tensor_add(t_s[:], t_d2[:], t_l2[:])
                t_r = tmp.tile([P, F], fio, tag="dif" if alias_tmps else "r")
                nc.scalar.activation(
                    t_r[:], t_s[:], Act.Sqrt, scale=(1.0 if prescale else 0.25)
                )

                if prescale:
                    t_hs = t_sum
                else:
                    # hs = 0.5*sum (tensor_scalar: 4x perf mode)
                    if hs_inplace:
                        t_hs = t_sum
                    else:
                        t_hs = tmp.tile([P, F], fio, tag="hs")
                    nc.vector.tensor_scalar_mul(t_hs[:], t_sum[:], 0.5)

                if packed == "tiled":
                    t_eo = outs.tile([P, 2 * F], fio, tag="eo")
                    nc.vector.tensor_sub(t_eo[:, 0:F], t_hs[:], t_r[:])
                    e1_eng.tensor_add(t_eo[:, F : 2 * F], t_hs[:], t_r[:])
                    st_eng.dma_start(eoutv[:, 2 * f0 : 2 * f0 + 2 * F], t_eo[:])
                elif packed:
                    t_eo = outs.tile([P, 2, F], fio, tag="eo")
                    nc.vector.tensor_sub(t_eo[:, 0, :], t_hs[:], t_r[:])
                    e1_eng.tensor_add(t_eo[:, 1, :], t_hs[:], t_r[:])
                    st_eng.dma_start(eoutv[:, :, sl], t_eo[:])
                else:
                    t_e0 = outs.tile([P, F], fio, tag="e0")
                    nc.vector.tensor_sub(t_e0[:], t_hs[:], t_r[:])
                    t_e1 = outs.tile([P, F], fio, tag="e1")
                    e1_eng.tensor_add(t_e1[:], t_hs[:], t_r[:])
                    store_eng.dma_start(e0v[:, sl], t_e0[:])
                    e1_store_eng.dma_start(e1v[:, sl], t_e1[:])

                f0 += F
    with _pin_act_table():
        nc.compile()
    return nc


def _get_nc(rows, **cfg):
    for k in ("ramp", "ramp_end", "load_queues", "store_queues"):
        if k in cfg:
            cfg[k] = tuple(cfg[k])
    key = (rows, tuple(sorted(cfg.items())))
    if key not in _cache:
        _cache[key] = _build(rows, **cfg)
    return _cache[key]


def kernel(d0, d1, lam, _trace=False, **cfg):
    np_io = np.dtype(cfg.get("io_dtype", "float16"))
    # prescale: ship d/2 and lam/2 (exact power-of-2 scale, folded into the
    # fp16 quantization) so the device computes m = d0'+d1' without a
    # separate 0.5x pass.
    sc = np.float32(0.5) if cfg.get("prescale", True) else np.float32(1.0)
    d0 = (np.asarray(d0, dtype=np.float32).ravel() * sc).astype(np_io)
    d1 = (np.asarray(d1, dtype=np.float32).ravel() * sc).astype(np_io)
    lam = (np.asarray(lam, dtype=np.float32).ravel() * sc).astype(np_io)
    n = d0.shape[0]

    # Per-core sample count: multiple of 128, cores cover ceil(n / 8).
    rows = -(-n // (N_CORES * P))  # ceil
    C = P * rows
    total = N_CORES * C
    pad = total - n
    if pad:
        z = np.zeros(pad, np_io)
        d0 = np.concatenate([d0, z])
        d1 = np.concatenate([d1, z])
        lam = np.concatenate([lam, z])

    packed = cfg.get("packed", "tiled")
    if packed == "tiled":
        # per core: per-partition, per-tile contiguous [3, F_i] blocks
        sizes = _tile_schedule(
            rows,
            cfg.get("f_tile", 2016),
            tuple(cfg.get("ramp", ())),
            tuple(cfg.get("ramp_end", (512,))),
        )
        in_maps = []
        for c in range(N_CORES):
            blk = np.empty((P, 3 * rows), np_io)
            d0c = d0[c * C : (c + 1) * C].reshape(P, rows)
            d1c = d1[c * C : (c + 1) * C].reshape(P, rows)
            lamc = lam[c * C : (c + 1) * C].reshape(P, rows)
            f0 = 0
            for F in sizes:
                b = 3 * f0
                blk[:, b : b + F] = d0c[:, f0 : f0 + F]
                blk[:, b + F : b + 2 * F] = d1c[:, f0 : f0 + F]
                blk[:, b + 2 * F : b + 3 * F] = lamc[:, f0 : f0 + F]
                f0 += F
            in_maps.append({"din": blk.ravel()})
    elif packed:
        # per core: [P, 3, rows] with [:, 0]=d0, [:, 1]=d1, [:, 2]=lam
        in_maps = []
        for c in range(N_CORES):
            blk = np.empty((P, 3, rows), np_io)
            blk[:, 0, :] = d0[c * C : (c + 1) * C].reshape(P, rows)
            blk[:, 1, :] = d1[c * C : (c + 1) * C].reshape(P, rows)
            blk[:, 2, :] = lam[c * C : (c + 1) * C].reshape(P, rows)
            in_maps.append({"din": blk.ravel()})
    else:
        in_maps = [
            {
                "d0": np.ascontiguousarray(d0[c * C : (c + 1) * C]),
                "d1": np.ascontiguousarray(d1[c * C : (c + 1) * C]),
                "lam": np.ascontiguousarray(lam[c * C : (c + 1) * C]),
            }
            for c in range(N_CORES)
        ]

    nc = _get_nc(rows, **cfg)
    res = run_bass_kernel_spmd(
        nc, in_maps, core_ids=list(range(N_CORES)), trace=_trace
    )
    global last_results
    last_results = res
    if packed == "tiled":
        full_e0 = np.empty((N_CORES, P, rows), np_io)
        full_e1 = np.empty((N_CORES, P, rows), np_io)
        for c in range(N_CORES):
            eo = res.results[c]["eout"].reshape(P, 2 * rows)
            f0 = 0
            for F in sizes:
                b = 2 * f0
                full_e0[c, :, f0 : f0 + F] = eo[:, b : b + F]
                full_e1[c, :, f0 : f0 + F] = eo[:, b + F : b + 2 * F]
                f0 += F
        full_e0 = full_e0.reshape(-1)
        full_e1 = full_e1.reshape(-1)
    elif packed:
        eo = np.stack(
            [res.results[c]["eout"].reshape(P, 2, rows) for c in range(N_CORES)]
        )  # [N_CORES, P, 2, rows]
        full_e0 = eo[:, :, 0, :].reshape(-1)
        full_e1 = eo[:, :, 1, :].reshape(-1)
    else:
        full_e0 = np.concatenate([res.results[c]["e0"] for c in range(N_CORES)])
        full_e1 = np.concatenate([res.results[c]["e1"] for c in range(N_CORES)])
    out = np.empty((n, 2), dtype=np.float32)
    out[:, 0] = full_e0[:n]
    out[:, 1] = full_e1[:n]
    return out


last_results = None
